# revision 31
# baseline (speedup 1.0000x reference)
"""MoE gating-network Bass kernel for 8 Trainium2 NeuronCores.

Data-parallel over the flattened token axis: hidden_states (4,4096,2048)
-> flat (16384,2048) -> 8 shards of (2048,2048), one per core.

The kernel is HBM-bandwidth-bound: per core it must stream the token
shard in, do a (2048 x 2048) @ (2048 x 64) matmul, and ship 2048x64
logits back.  To halve the stream, tokens are shipped as fp16 (8 MB
instead of 16 MB per core) and the device computes RAW expert dots
(sim columns pre-normalized, pre-masked, scaled by 64 to dodge fp16
subnormals) with fp32 PSUM accumulation, shipped back as bf16.
Everything else happens on the host:

  logits = rawT.T * (rnorm / 64)          rnorm = 1/max(||x||_fp32, eps)
  mask   = logits > gates*sigmoid(T)      (+ reference top-k fallback)

fp16 rounding perturbs a cosine logit by at most ~2*2^-11 in the worst
case (Cauchy-Schwarz: |sum s_i*eps_i*x_i| <= eps*||s||*||x||, then
/||x||) and ~1e-5 rms in practice; the bf16 result rounding adds
< 4e-5.  Every logit within TAU=3e-4 of a decision boundary (0 or the
gate threshold) is therefore recomputed exactly on host in fp64 (~10k
of the 1M entries) and patched into both outputs, so the activation
mask is exact and the logits match fp32 reference to ~1e-3 absolute
worst-case.

Per-core device program (hand-scheduled raw Bass; this walrus build
supports only ONE embedded sync wait per instruction, so cross-engine
deps are standalone wait_ge ops).  The two HWDGE rings drain 1:1 per
descriptor when both have work, so ring bytes are balanced exactly
(4.125 MB each) so that contraction chunks land in consumption order
and only two matmuls + copies + output DMAs trail the final bytes:

  SP ring : half of simn, chunk pairs {0,1}{4,5}{8,9}, low token-
            halves of chunks 12..15, bank-0 output DMA
  ACT ring: other half of simn, pairs {2,3}{6,7}{10,11}, high token-
            halves of chunks 12..15, bank-1 output DMA
  PE      : HAM warm-up, then 64 sim-stationary fp16 matmuls (4 token
            groups x 16 chunks; the two PE column halves run
            concurrently via tile_position); for the lo/hi-split tail
            chunks the bank-0 groups run as soon as the low half lands
  DVE     : PSUM bank0 -> SBUF bf16;  ACT: PSUM bank1 -> SBUF bf16
            (one engine per bank: two engines must not touch the same
            PSUM bank concurrently; ACT pre-loads its activation table)

Returns raw logitsT staged as (128, 1024) bf16 per core; the host
unscrambles (expert, bank, token) -> (token, expert).
"""

import numpy as np

# Hardcoded problem shapes (kernel.py must be self-contained).
B, T, C, E = 4, 4096, 2048, 64
N = B * T
N_CORES = 8
NS = N // N_CORES          # tokens per core (2048)
P = 128                    # partitions
KC = C // P                # contraction chunks (16)
TW = 512                   # tokens per matmul group (one PSUM bank)
NG = NS // TW              # token groups per core (4)
NMM = KC * NG              # real matmuls per core (64)
C0 = NMM - 2               # sMM count at which PSUM bank 0 is complete
NWARM = 12                 # HAM warm-up matmuls
SSCALE = 64.0              # sim-matrix scale (fp16 subnormal guard)
EPS = 1e-12
TAU = 3e-4                 # host exact-repair band around decision boundaries

# (ring, sem_target, chunks) per input DMA.  Both rings carry exactly
# 4.125 MB (half of simn + 3 chunk pairs + token-halves of chunks
# 12..15), so the two HWDGE rings - which drain 1:1 per descriptor -
# exhaust together and chunks land in consumption order; each of the
# last four chunks arrives as a mirrored lo/hi token-half per ring
# ("lo" = tokens 0..1023 = PSUM bank-0 groups).
DMA_PLAN = [
    ("e", 16, (0, 1)),
    ("o", 16, (2, 3)),
    ("e", 32, (4, 5)),
    ("o", 32, (6, 7)),
    ("e", 48, (8, 9)),
    ("o", 48, (10, 11)),
    ("e", 64, (12, "lo")),
    ("o", 64, (12, "hi")),
    ("e", 80, (13, "lo")),
    ("o", 80, (13, "hi")),
    ("e", 96, (14, "lo")),
    ("o", 96, (14, "hi")),
    ("e", 112, (15, "lo")),
    ("o", 112, (15, "hi")),
]
JUNK_BEFORE = {14: 1, 15: 1}  # keep-alive matmuls before these chunk waits
JSPLIT = 12                   # chunks >= JSPLIT arrive as lo/hi halves
SH_SPLIT = KC * E // 2        # sh columns via SP ring; rest via ACT ring
# chunk -> (ring, sem count) for the paired-chunk PE waits
CHUNK_WAIT = {}
for _ring, _cnt, _chunks in DMA_PLAN:
    if len(_chunks) == 2 and _chunks[1] in ("lo", "hi"):
        continue
    for _j in _chunks:
        CHUNK_WAIT[_j] = (_ring, _cnt)


def _np_reference(flat, sim_matrix, gates, temperature, experts_mask, k):
    """Reference math in numpy - correctness fallback path."""
    fn = flat / np.maximum(np.linalg.norm(flat, axis=-1, keepdims=True), EPS)
    sn = sim_matrix / np.maximum(
        np.linalg.norm(sim_matrix, axis=0, keepdims=True), EPS
    )
    logits = (fn @ sn) * experts_mask
    logit_scale = 1.0 / (1.0 + np.exp(-temperature[0]))
    gated = np.maximum(logits - gates * logit_scale, 0.0)
    hard = (gated > 0).astype(np.float32)
    inactive = hard.sum(axis=1) == 0
    topk_idx = np.argsort(-logits, axis=1)[:, :k]
    fallback = np.zeros_like(logits)
    np.put_along_axis(fallback, topk_idx, 1.0, axis=1)
    mask = np.where(inactive[:, None], fallback, hard)
    return mask.astype(np.float32), logits.astype(np.float32)


def build_bass():
    """Build the per-core Bass program (identical on all 8 cores)."""
    from contextlib import ExitStack

    import concourse.bass as bass
    from concourse import mybir

    f16 = mybir.dt.float16
    bf16 = mybir.dt.bfloat16
    f32 = mybir.dt.float32

    nc = bass.Bass(
        "TRN2",
        target_bir_lowering=False,
        debug=False,
        enable_asserts=False,
        num_devices=1,
        detect_race_conditions=False,
    )
    xh = nc.dram_tensor("xh", [C, NS], f16, kind="ExternalInput").ap()
    sh = nc.dram_tensor("sh", [P, KC * E], f16, kind="ExternalInput").ap()
    raw_o = nc.dram_tensor("raw", [P, 2 * TW], bf16, kind="ExternalOutput").ap()

    xv = xh.rearrange("(j p) t -> j p t", p=P)          # (KC, P, NS)
    xv2 = xh.rearrange("(pi q p) t -> pi p q t", q=2, p=P)  # (8, P, 2, NS)

    with ExitStack() as ctx:
        ec = ctx.enter_context

        dXe = ec(nc.semaphore("dXe"))  # SP-ring x^T DMAs
        dXo = ec(nc.semaphore("dXo"))  # ACT-ring x^T DMAs
        dCs = ec(nc.semaphore("dCs"))  # simn DMA
        sW = ec(nc.semaphore("sW"))    # scratch memsets (DVE)
        sMM = ec(nc.semaphore("sMM"))  # real matmuls done (PE)
        sCa = ec(nc.semaphore("sCa"))  # PSUM bank0 copied (DVE)
        sCb = ec(nc.semaphore("sCb"))  # PSUM bank1 copied (ACT)
        dO = ec(nc.semaphore("dO"))    # output DMAs

        xh_all = ec(nc.sbuf_tensor("xh_all", [P, KC, NS], f16))
        sh_sb = ec(nc.sbuf_tensor("sh_sb", [P, KC * E], f16))
        wj = ec(nc.sbuf_tensor("wj", [P, 2 * P], f16))      # warm-up junk
        tdum = ec(nc.sbuf_tensor("tdum", [P, 8], f32))      # ACT table dummy
        out_sb = ec(nc.sbuf_tensor("out_sb", [P, 2, TW], bf16))

        # Token group g accumulates in PSUM bank g//2, partitions
        # 64*(g%2) .. 64*(g%2)+64 (PE column-group tiling: the two
        # halves of the PE array run concurrently).
        plt = ec(nc.psum_tensor("plt", [P, 2, TW], f32))    # 2 banks
        pw = ec(nc.psum_tensor("pw", [P, TW], f32))         # warm-up bank

        block = ec(nc.Block())

        def dma_in(eng, sem, chunks):
            if len(chunks) == 2 and chunks[1] in ("lo", "hi"):
                j = chunks[0]
                lo = 0 if chunks[1] == "lo" else NS // 2
                eng.dma_start(
                    out=xh_all[:, j, lo : lo + NS // 2],
                    in_=xv[j][:, lo : lo + NS // 2],
                ).then_inc(sem, 16)
            elif len(chunks) == 2:
                eng.dma_start(
                    out=xh_all[:, chunks[0] : chunks[0] + 2, :],
                    in_=xv2[chunks[0] // 2],
                ).then_inc(sem, 16)
            else:
                eng.dma_start(
                    out=xh_all[:, chunks[0], :], in_=xv[chunks[0]]
                ).then_inc(sem, 16)

        # --- SP ring: sh head + its chunk DMAs + bank0 output --------------
        @block.sync
        def _(sync):
            sync.dma_start(
                out=sh_sb[:, :SH_SPLIT], in_=sh[:, :SH_SPLIT]
            ).then_inc(dCs, 16)
            for ring, _cnt, chunks in DMA_PLAN:
                if ring == "e":
                    dma_in(sync, dXe, chunks)
            sync.wait_ge(sCa, 1)
            sync.dma_start(out=raw_o[:, 0:TW], in_=out_sb[:, 0, :]).then_inc(
                dO, 16
            )
            sync.wait_ge(dO, 32)

        # --- ACT ring: sh tail + its chunk DMAs; bank1 copy + output -------
        @block.scalar
        def _(scalar):
            scalar.dma_start(
                out=sh_sb[:, SH_SPLIT:], in_=sh[:, SH_SPLIT:]
            ).then_inc(dCs, 16)
            for ring, _cnt, chunks in DMA_PLAN:
                if ring == "o":
                    dma_in(scalar, dXo, chunks)
            # Pre-load the activation table (first ACT op pays ~1us).
            scalar.wait_ge(sW, 2)
            scalar.copy(out=tdum[:, 4:8], in_=tdum[:, 0:4])
            scalar.wait_ge(sMM, NMM)
            scalar.copy(out=out_sb[:, 1, :], in_=plt[:, 1, :]).then_inc(
                sCb, 1
            )
            scalar.dma_start(
                out=raw_o[:, TW : 2 * TW], in_=out_sb[:, 1, :]
            ).then_inc(dO, 16)

        # --- PE: warm-up + sim-stationary fp16 matmuls ---------------------
        @block.tensor
        def _(tensor):
            def mm(j, g):
                half = g % 2
                return tensor.matmul(
                    plt[E * half : E * (half + 1), g // 2, :],
                    sh_sb[:, j * E : (j + 1) * E],
                    xh_all[:, j, g * TW : (g + 1) * TW],
                    start=(j == 0),
                    stop=(j == KC - 1),
                    tile_position=(0, E * half),
                    # per-element has_written bits make partition-
                    # disjoint groups in one bank safe; the sim check
                    # is bank-level
                    skip_group_check=True,
                ).then_inc(sMM, 1)

            tensor.wait_ge(sW, 1)
            for _ in range(NWARM):
                tensor.matmul(
                    pw[:, :P], wj[:, 0:P], wj[:, P : 2 * P],
                    start=True, stop=True,
                )
            def junk(n):
                for _ in range(n):
                    tensor.matmul(
                        pw[:, :P], wj[:, 0:P], wj[:, P : 2 * P],
                        start=True, stop=True,
                    )

            tensor.wait_ge(dCs, 32)
            last = (None, 0)
            for j in range(JSPLIT):
                junk(JUNK_BEFORE.get(j, 0))
                if CHUNK_WAIT[j] != last:
                    last = CHUNK_WAIT[j]
                    tensor.wait_ge(dXe if last[0] == "e" else dXo, last[1])
                for g in range(NG):
                    mm(j, g)
            # Chunks 12..15 arrive as mirrored token-halves, one per ring:
            # bank-0 groups run off the low half as soon as it lands, so
            # chunk 15's bank-0 copy/output overlaps its bank-1 groups.
            for j in range(JSPLIT, KC):
                junk(JUNK_BEFORE.get(j, 0))
                cnt = 64 + 16 * (j - JSPLIT)
                tensor.wait_ge(dXe, cnt)
                mm(j, 0)
                mm(j, 1)
                tensor.wait_ge(dXo, cnt)
                mm(j, 2)
                mm(j, 3)

        # --- DVE: scratch memsets + bank0 copy -----------------------------
        @block.vector
        def _(vector):
            vector.memset(wj[:], 0.25).then_inc(sW, 1)
            vector.memset(tdum[:], 0.0).then_inc(sW, 1)
            # Bank0 groups complete at sMM=C0 while bank-1 groups still
            # run - different PSUM bank, concurrent access is safe.
            vector.wait_ge(sMM, C0)
            vector.tensor_scalar_mul(
                out=out_sb[:, 0, :], in0=plt[:, 0, :], scalar1=1.0
            ).then_inc(sCa, 1)

    return nc


_NC_CACHE = {}


def _get_nc():
    if "nc" not in _NC_CACHE:
        _NC_CACHE["nc"] = build_bass()
    return _NC_CACHE["nc"]


def _prep_inputs(hidden_states, sim_matrix, gates, temperature, experts_mask):
    flat = np.ascontiguousarray(
        np.asarray(hidden_states, dtype=np.float32).reshape(N, C)
    )
    sim_matrix = np.asarray(sim_matrix, dtype=np.float32)
    gates = np.asarray(gates, dtype=np.float32)
    temperature = np.asarray(temperature, dtype=np.float32)
    em = np.asarray(experts_mask, dtype=np.float32)

    # Per-shard fp16 channel-major tokens.
    f16 = flat.astype(np.float16)
    xhs = [
        np.ascontiguousarray(f16[c * NS : (c + 1) * NS].T)
        for c in range(N_CORES)
    ]

    # Column-normalized sim matrix in fp64 (reused by the exact repair),
    # masked + scaled for the device.
    sim64 = sim_matrix.astype(np.float64)
    sn64 = sim64 / np.maximum(np.sqrt((sim64 * sim64).sum(axis=0)), EPS)
    sh_dev = (sn64 * em[None, :] * SSCALE).astype(np.float16)  # (C, E)
    sh_dev = np.ascontiguousarray(
        sh_dev.reshape(KC, P, E).transpose(1, 0, 2).reshape(P, KC * E)
    )
    shs = [sh_dev] * N_CORES

    # Per-token inverse norms.  fp32 einsum is plenty: norm error is a
    # tiny positive relative scale - it cannot flip a logit sign, and
    # flagged entries are recomputed with exact fp64 norms anyway.
    ssq = np.einsum("nc,nc->n", flat, flat)
    rnorm = 1.0 / np.maximum(np.sqrt(ssq.astype(np.float64)), EPS)  # fp64

    hctx = {
        "flat": flat,
        "sn64": sn64,
        "rnorm": rnorm,
        "gates": gates,
        "temperature": temperature,
        "em": em,
    }
    return xhs, shs, hctx


def _decode_raw(raws):
    """(cores, 128, 1024) bf16 -> (N, E) raw expert dots (fp32)."""
    lg = np.empty((N, E), dtype=np.float32)
    for c in range(N_CORES):
        r = np.asarray(raws[c], dtype=np.float32).reshape(2, E, 2, TW)
        base = c * NS
        for g in range(NG):
            lg[base + g * TW : base + (g + 1) * TW, :] = r[g % 2, :, g // 2, :].T
    return lg


def run_on_device(xhs, shs, hctx, k, trace=False):
    import ml_dtypes  # noqa: F401  (bf16 numpy dtype registration)
    from concourse.bass_utils import run_bass_kernel_spmd

    nc = _get_nc()
    in_maps = [{"xh": xhs[c], "sh": shs[c]} for c in range(N_CORES)]
    if "warmed" not in _NC_CACHE:
        # The very first NEFF execution in a process runs with unreliable
        # semaphore state (engines free-run past their waits and read
        # uninitialized SBUF on some cores), so its outputs are garbage.
        # Execute once and discard; all later executions are bit-stable.
        run_bass_kernel_spmd(
            nc, in_maps, core_ids=list(range(N_CORES)), trace=False
        )
        _NC_CACHE["warmed"] = True
    res = run_bass_kernel_spmd(
        nc, in_maps, core_ids=list(range(N_CORES)), trace=trace
    )
    raws = [res.results[c]["raw"] for c in range(N_CORES)]
    rawN = _decode_raw(raws)  # (N, E) raw dots (scaled by SSCALE)

    flat = hctx["flat"]
    sn64 = hctx["sn64"]
    rnorm = hctx["rnorm"]
    gates = hctx["gates"]
    em = hctx["em"]

    logits = (rawN.astype(np.float64) * (rnorm / SSCALE)[:, None]).astype(
        np.float32
    )

    logit_scale = 1.0 / (1.0 + np.exp(-float(hctx["temperature"][0])))
    thresh = (gates * logit_scale).astype(np.float32)  # (E,)

    # Exact fp64 repair of every logit near a decision boundary.  fp16
    # rounding moves a logit by < TAU (see module docstring), so any
    # entry whose sign (vs 0 or vs the gate threshold) could disagree
    # with fp32 reference lies inside this band.
    band = (np.abs(logits) < TAU) | (np.abs(logits - thresh[None, :]) < TAU)
    band &= em[None, :] != 0
    rows, cols = np.nonzero(band)
    if rows.size:
        ur, inv = np.unique(rows, return_inverse=True)
        xu = flat[ur].astype(np.float64)  # (nu, C) exact fp64 rows
        rn_u = 1.0 / np.maximum(np.sqrt((xu * xu).sum(axis=1)), EPS)
        d = np.einsum("ij,ij->i", xu[inv], sn64[:, cols].T)
        exact = d * rn_u[inv] * em[cols]
        logits[rows, cols] = exact.astype(np.float32)

    # Reference tail in numpy from the repaired logits.
    gated = logits - thresh[None, :]
    hard = (gated > 0).astype(np.float32)
    inactive = hard.sum(axis=1) == 0
    mask = hard
    if inactive.any():
        li = logits[inactive]
        topk_idx = np.argsort(-li, axis=1, kind="stable")[:, :k]
        fallback = np.zeros_like(li)
        np.put_along_axis(fallback, topk_idx, 1.0, axis=1)
        mask[inactive] = fallback
    return mask, logits, res


def kernel(hidden_states, sim_matrix, gates, temperature, experts_mask,
           min_experts_per_tok):
    k = int(np.asarray(min_experts_per_tok))
    if not (1 <= k <= E):
        flat = np.asarray(hidden_states, dtype=np.float32).reshape(N, C)
        return _np_reference(
            flat,
            np.asarray(sim_matrix, dtype=np.float32),
            np.asarray(gates, dtype=np.float32),
            np.asarray(temperature, dtype=np.float32),
            np.asarray(experts_mask, dtype=np.float32),
            k,
        )
    xhs, shs, hctx = _prep_inputs(
        hidden_states, sim_matrix, gates, temperature, experts_mask
    )
    mask, logits, _ = run_on_device(xhs, shs, hctx, k)
    return mask, logits


# revision 35
# speedup vs baseline: 1.0063x; 1.0063x over previous
"""MoE gating-network Bass kernel for 8 Trainium2 NeuronCores.

Data-parallel over the flattened token axis: hidden_states (4,4096,2048)
-> flat (16384,2048) -> 8 shards of (2048,2048), one per core.

The kernel is HBM-bandwidth-bound: per core it must stream the token
shard in, do a (2048 x 2048) @ (2048 x 64) matmul, and ship 2048x64
logits back.  To halve the stream, tokens are shipped as fp16 (8 MB
instead of 16 MB per core) and the device computes RAW expert dots
(sim columns pre-normalized, pre-masked, scaled by 64 to dodge fp16
subnormals) with fp32 PSUM accumulation, shipped back as bf16.
Everything else happens on the host:

  logits = rawT.T * (rnorm / 64)          rnorm = 1/max(||x||_fp32, eps)
  mask   = logits > gates*sigmoid(T)      (+ reference top-k fallback)

fp16 rounding perturbs a cosine logit by at most ~2*2^-11 in the worst
case (Cauchy-Schwarz: |sum s_i*eps_i*x_i| <= eps*||s||*||x||, then
/||x||) and ~1e-5 rms in practice; the bf16 result rounding adds
< 4e-5.  Every logit within TAU=3e-4 of a decision boundary (0 or the
gate threshold) is therefore recomputed exactly on host in fp64 (~10k
of the 1M entries) and patched into both outputs, so the activation
mask is exact and the logits match fp32 reference to ~1e-3 absolute
worst-case.

Per-core device program (hand-scheduled raw Bass; this walrus build
supports only ONE embedded sync wait per instruction, so cross-engine
deps are standalone wait_ge ops).  The two HWDGE rings drain 1:1 per
descriptor when both have work, so ring bytes are balanced exactly
(4.125 MB each) so that contraction chunks land in consumption order
and only two matmuls + copies + output DMAs trail the final bytes:

  SP ring : half of simn, chunk pairs {0,1}{4,5}{8,9}, low token-
            halves of chunks 12..15, bank-0 output DMA
  ACT ring: other half of simn, pairs {2,3}{6,7}{10,11}, high token-
            halves of chunks 12..15, bank-1 output DMA
  PE      : HAM warm-up, then 64 sim-stationary fp16 matmuls (4 token
            groups x 16 chunks; the two PE column halves run
            concurrently via tile_position); for the lo/hi-split tail
            chunks the bank-0 groups run as soon as the low half lands
  DVE     : PSUM bank0 -> SBUF bf16;  ACT: PSUM bank1 -> SBUF bf16
            (one engine per bank: two engines must not touch the same
            PSUM bank concurrently; ACT pre-loads its activation table)

Returns raw logitsT staged as (128, 1024) bf16 per core; the host
unscrambles (expert, bank, token) -> (token, expert).
"""

import numpy as np

# Hardcoded problem shapes (kernel.py must be self-contained).
B, T, C, E = 4, 4096, 2048, 64
N = B * T
N_CORES = 8
NS = N // N_CORES          # tokens per core (2048)
P = 128                    # partitions
KC = C // P                # contraction chunks (16)
TW = 512                   # tokens per matmul group (one PSUM bank)
NG = NS // TW              # token groups per core (4)
NMM = KC * NG              # real matmuls per core (64)
C0 = NMM - 2               # sMM count at which PSUM bank 0 is complete
NWARM = 12                 # HAM warm-up matmuls
SSCALE = 64.0              # sim-matrix scale (fp16 subnormal guard)
EPS = 1e-12
TAU = 3e-4                 # host exact-repair band around decision boundaries

# (ring, sem_target, chunks) per input DMA.  simn rides the third
# (gpsimd SWDGE) queue, so both HWDGE rings - which drain 1:1 per
# descriptor - carry exactly 4.0 MB of chunk data and exhaust
# together; each ring leads with a small single-chunk DMA so its
# doorbell rings early, chunks land in consumption order, and chunks
# 14/15 arrive as mirrored lo/hi token-halves per ring ("lo" =
# tokens 0..1023 = PSUM bank-0 groups), with chunk 15 last.
DMA_PLAN = [
    ("e", 16, (0,)),
    ("o", 16, (1,)),
    ("e", 32, (2, 3)),
    ("o", 32, (4, 5)),
    ("e", 48, (6, 7)),
    ("o", 48, (8, 9)),
    ("e", 64, (10, 11)),
    ("o", 64, (12,)),
    ("o", 80, (13,)),
    ("e", 80, (14, "lo")),
    ("o", 96, (14, "hi")),
    ("e", 96, (15, "lo")),
    ("o", 112, (15, "hi")),
]
JUNK_BEFORE = {14: 1, 15: 1}  # keep-alive matmuls before these chunk waits
JSPLIT = 14                   # chunks >= JSPLIT arrive as lo/hi halves
SPLIT_WAITS = {14: (80, 96), 15: (96, 112)}  # (dXe lo, dXo hi) counts
# chunk -> (ring, sem count) for the whole-chunk PE waits
CHUNK_WAIT = {}
for _ring, _cnt, _chunks in DMA_PLAN:
    if len(_chunks) == 2 and _chunks[1] in ("lo", "hi"):
        continue
    for _j in _chunks:
        CHUNK_WAIT[_j] = (_ring, _cnt)


def _np_reference(flat, sim_matrix, gates, temperature, experts_mask, k):
    """Reference math in numpy - correctness fallback path."""
    fn = flat / np.maximum(np.linalg.norm(flat, axis=-1, keepdims=True), EPS)
    sn = sim_matrix / np.maximum(
        np.linalg.norm(sim_matrix, axis=0, keepdims=True), EPS
    )
    logits = (fn @ sn) * experts_mask
    logit_scale = 1.0 / (1.0 + np.exp(-temperature[0]))
    gated = np.maximum(logits - gates * logit_scale, 0.0)
    hard = (gated > 0).astype(np.float32)
    inactive = hard.sum(axis=1) == 0
    topk_idx = np.argsort(-logits, axis=1)[:, :k]
    fallback = np.zeros_like(logits)
    np.put_along_axis(fallback, topk_idx, 1.0, axis=1)
    mask = np.where(inactive[:, None], fallback, hard)
    return mask.astype(np.float32), logits.astype(np.float32)


def build_bass():
    """Build the per-core Bass program (identical on all 8 cores)."""
    from contextlib import ExitStack

    import concourse.bass as bass
    from concourse import mybir

    f16 = mybir.dt.float16
    bf16 = mybir.dt.bfloat16
    f32 = mybir.dt.float32

    nc = bass.Bass(
        "TRN2",
        target_bir_lowering=False,
        debug=False,
        enable_asserts=False,
        num_devices=1,
        detect_race_conditions=False,
    )
    xh = nc.dram_tensor("xh", [C, NS], f16, kind="ExternalInput").ap()
    sh = nc.dram_tensor("sh", [P, KC * E], f16, kind="ExternalInput").ap()
    raw_o = nc.dram_tensor("raw", [P, 2 * TW], bf16, kind="ExternalOutput").ap()

    xv = xh.rearrange("(j p) t -> j p t", p=P)          # (KC, P, NS)
    xv2 = xh.rearrange("(pi q p) t -> pi p q t", q=2, p=P)  # (8, P, 2, NS)

    with ExitStack() as ctx:
        ec = ctx.enter_context

        dXe = ec(nc.semaphore("dXe"))  # SP-ring x^T DMAs
        dXo = ec(nc.semaphore("dXo"))  # ACT-ring x^T DMAs
        dCs = ec(nc.semaphore("dCs"))  # simn DMA
        sW = ec(nc.semaphore("sW"))    # scratch memsets (DVE)
        sMM = ec(nc.semaphore("sMM"))  # real matmuls done (PE)
        sCa = ec(nc.semaphore("sCa"))  # PSUM bank0 copied (DVE)
        sCb = ec(nc.semaphore("sCb"))  # PSUM bank1 copied (ACT)
        dO = ec(nc.semaphore("dO"))    # output DMAs

        xh_all = ec(nc.sbuf_tensor("xh_all", [P, KC, NS], f16))
        sh_sb = ec(nc.sbuf_tensor("sh_sb", [P, KC * E], f16))
        wj = ec(nc.sbuf_tensor("wj", [P, 2 * P], f16))      # warm-up junk
        tdum = ec(nc.sbuf_tensor("tdum", [P, 8], f32))      # ACT table dummy
        out_sb = ec(nc.sbuf_tensor("out_sb", [P, 2, TW], bf16))

        # Token group g accumulates in PSUM bank g//2, partitions
        # 64*(g%2) .. 64*(g%2)+64 (PE column-group tiling: the two
        # halves of the PE array run concurrently).
        plt = ec(nc.psum_tensor("plt", [P, 2, TW], f32))    # 2 banks
        pw = ec(nc.psum_tensor("pw", [P, TW], f32))         # warm-up bank

        block = ec(nc.Block())

        def dma_in(eng, sem, chunks):
            if len(chunks) == 2 and chunks[1] in ("lo", "hi"):
                j = chunks[0]
                lo = 0 if chunks[1] == "lo" else NS // 2
                eng.dma_start(
                    out=xh_all[:, j, lo : lo + NS // 2],
                    in_=xv[j][:, lo : lo + NS // 2],
                ).then_inc(sem, 16)
            elif len(chunks) == 2:
                eng.dma_start(
                    out=xh_all[:, chunks[0] : chunks[0] + 2, :],
                    in_=xv2[chunks[0] // 2],
                ).then_inc(sem, 16)
            else:
                eng.dma_start(
                    out=xh_all[:, chunks[0], :], in_=xv[chunks[0]]
                ).then_inc(sem, 16)

        # --- SP ring: its chunk DMAs + bank0 output ------------------------
        @block.sync
        def _(sync):
            for ring, _cnt, chunks in DMA_PLAN:
                if ring == "e":
                    dma_in(sync, dXe, chunks)
            sync.wait_ge(sCa, 1)
            sync.dma_start(out=raw_o[:, 0:TW], in_=out_sb[:, 0, :]).then_inc(
                dO, 16
            )
            sync.wait_ge(dO, 32)

        # --- GPSIMD (SWDGE queue): simn consts off the HWDGE rings ---------
        @block.gpsimd
        def _(gpsimd):
            gpsimd.dma_start(out=sh_sb[:], in_=sh[:]).then_inc(dCs, 16)

        # --- ACT ring: its chunk DMAs; bank1 copy + output -----------------
        @block.scalar
        def _(scalar):
            for ring, _cnt, chunks in DMA_PLAN:
                if ring == "o":
                    dma_in(scalar, dXo, chunks)
            # Pre-load the activation table (first ACT op pays ~1us).
            scalar.wait_ge(sW, 2)
            scalar.copy(out=tdum[:, 4:8], in_=tdum[:, 0:4])
            scalar.wait_ge(sMM, NMM)
            scalar.copy(out=out_sb[:, 1, :], in_=plt[:, 1, :]).then_inc(
                sCb, 1
            )
            scalar.dma_start(
                out=raw_o[:, TW : 2 * TW], in_=out_sb[:, 1, :]
            ).then_inc(dO, 16)

        # --- PE: warm-up + sim-stationary fp16 matmuls ---------------------
        @block.tensor
        def _(tensor):
            def mm(j, g):
                half = g % 2
                return tensor.matmul(
                    plt[E * half : E * (half + 1), g // 2, :],
                    sh_sb[:, j * E : (j + 1) * E],
                    xh_all[:, j, g * TW : (g + 1) * TW],
                    start=(j == 0),
                    stop=(j == KC - 1),
                    tile_position=(0, E * half),
                    # per-element has_written bits make partition-
                    # disjoint groups in one bank safe; the sim check
                    # is bank-level
                    skip_group_check=True,
                ).then_inc(sMM, 1)

            tensor.wait_ge(sW, 1)
            for _ in range(NWARM):
                tensor.matmul(
                    pw[:, :P], wj[:, 0:P], wj[:, P : 2 * P],
                    start=True, stop=True,
                )
            def junk(n):
                for _ in range(n):
                    tensor.matmul(
                        pw[:, :P], wj[:, 0:P], wj[:, P : 2 * P],
                        start=True, stop=True,
                    )

            tensor.wait_ge(dCs, 16)
            last = (None, 0)
            for j in range(JSPLIT):
                junk(JUNK_BEFORE.get(j, 0))
                if CHUNK_WAIT[j] != last:
                    last = CHUNK_WAIT[j]
                    tensor.wait_ge(dXe if last[0] == "e" else dXo, last[1])
                for g in range(NG):
                    mm(j, g)
            # Chunks 14/15 arrive as mirrored token-halves, one per ring:
            # bank-0 groups run off the low half as soon as it lands, so
            # chunk 15's bank-0 copy/output overlaps its bank-1 groups.
            for j in range(JSPLIT, KC):
                junk(JUNK_BEFORE.get(j, 0))
                lo_cnt, hi_cnt = SPLIT_WAITS[j]
                tensor.wait_ge(dXe, lo_cnt)
                mm(j, 0)
                mm(j, 1)
                tensor.wait_ge(dXo, hi_cnt)
                mm(j, 2)
                mm(j, 3)

        # --- DVE: scratch memsets + bank0 copy -----------------------------
        @block.vector
        def _(vector):
            vector.memset(wj[:], 0.25).then_inc(sW, 1)
            vector.memset(tdum[:], 0.0).then_inc(sW, 1)
            # Bank0 groups complete at sMM=C0 while bank-1 groups still
            # run - different PSUM bank, concurrent access is safe.
            vector.wait_ge(sMM, C0)
            vector.tensor_scalar_mul(
                out=out_sb[:, 0, :], in0=plt[:, 0, :], scalar1=1.0
            ).then_inc(sCa, 1)

    return nc


_NC_CACHE = {}


def _get_nc():
    if "nc" not in _NC_CACHE:
        _NC_CACHE["nc"] = build_bass()
    return _NC_CACHE["nc"]


def _prep_inputs(hidden_states, sim_matrix, gates, temperature, experts_mask):
    flat = np.ascontiguousarray(
        np.asarray(hidden_states, dtype=np.float32).reshape(N, C)
    )
    sim_matrix = np.asarray(sim_matrix, dtype=np.float32)
    gates = np.asarray(gates, dtype=np.float32)
    temperature = np.asarray(temperature, dtype=np.float32)
    em = np.asarray(experts_mask, dtype=np.float32)

    # Per-shard fp16 channel-major tokens.
    f16 = flat.astype(np.float16)
    xhs = [
        np.ascontiguousarray(f16[c * NS : (c + 1) * NS].T)
        for c in range(N_CORES)
    ]

    # Column-normalized sim matrix in fp64 (reused by the exact repair),
    # masked + scaled for the device.
    sim64 = sim_matrix.astype(np.float64)
    sn64 = sim64 / np.maximum(np.sqrt((sim64 * sim64).sum(axis=0)), EPS)
    sh_dev = (sn64 * em[None, :] * SSCALE).astype(np.float16)  # (C, E)
    sh_dev = np.ascontiguousarray(
        sh_dev.reshape(KC, P, E).transpose(1, 0, 2).reshape(P, KC * E)
    )
    shs = [sh_dev] * N_CORES

    # Per-token inverse norms.  fp32 einsum is plenty: norm error is a
    # tiny positive relative scale - it cannot flip a logit sign, and
    # flagged entries are recomputed with exact fp64 norms anyway.
    ssq = np.einsum("nc,nc->n", flat, flat)
    rnorm = 1.0 / np.maximum(np.sqrt(ssq.astype(np.float64)), EPS)  # fp64

    hctx = {
        "flat": flat,
        "sn64": sn64,
        "rnorm": rnorm,
        "gates": gates,
        "temperature": temperature,
        "em": em,
    }
    return xhs, shs, hctx


def _decode_raw(raws):
    """(cores, 128, 1024) bf16 -> (N, E) raw expert dots (fp32)."""
    lg = np.empty((N, E), dtype=np.float32)
    for c in range(N_CORES):
        r = np.asarray(raws[c], dtype=np.float32).reshape(2, E, 2, TW)
        base = c * NS
        for g in range(NG):
            lg[base + g * TW : base + (g + 1) * TW, :] = r[g % 2, :, g // 2, :].T
    return lg


def run_on_device(xhs, shs, hctx, k, trace=False):
    import ml_dtypes  # noqa: F401  (bf16 numpy dtype registration)
    from concourse.bass_utils import run_bass_kernel_spmd

    nc = _get_nc()
    in_maps = [{"xh": xhs[c], "sh": shs[c]} for c in range(N_CORES)]
    if "warmed" not in _NC_CACHE:
        # The very first NEFF execution in a process runs with unreliable
        # semaphore state (engines free-run past their waits and read
        # uninitialized SBUF on some cores), so its outputs are garbage.
        # Execute once and discard; all later executions are bit-stable.
        run_bass_kernel_spmd(
            nc, in_maps, core_ids=list(range(N_CORES)), trace=False
        )
        _NC_CACHE["warmed"] = True
    res = run_bass_kernel_spmd(
        nc, in_maps, core_ids=list(range(N_CORES)), trace=trace
    )
    raws = [res.results[c]["raw"] for c in range(N_CORES)]
    rawN = _decode_raw(raws)  # (N, E) raw dots (scaled by SSCALE)

    flat = hctx["flat"]
    sn64 = hctx["sn64"]
    rnorm = hctx["rnorm"]
    gates = hctx["gates"]
    em = hctx["em"]

    logits = (rawN.astype(np.float64) * (rnorm / SSCALE)[:, None]).astype(
        np.float32
    )

    logit_scale = 1.0 / (1.0 + np.exp(-float(hctx["temperature"][0])))
    thresh = (gates * logit_scale).astype(np.float32)  # (E,)

    # Exact fp64 repair of every logit near a decision boundary.  fp16
    # rounding moves a logit by < TAU (see module docstring), so any
    # entry whose sign (vs 0 or vs the gate threshold) could disagree
    # with fp32 reference lies inside this band.
    band = (np.abs(logits) < TAU) | (np.abs(logits - thresh[None, :]) < TAU)
    band &= em[None, :] != 0
    rows, cols = np.nonzero(band)
    if rows.size:
        ur, inv = np.unique(rows, return_inverse=True)
        xu = flat[ur].astype(np.float64)  # (nu, C) exact fp64 rows
        rn_u = 1.0 / np.maximum(np.sqrt((xu * xu).sum(axis=1)), EPS)
        d = np.einsum("ij,ij->i", xu[inv], sn64[:, cols].T)
        exact = d * rn_u[inv] * em[cols]
        logits[rows, cols] = exact.astype(np.float32)

    # Reference tail in numpy from the repaired logits.
    gated = logits - thresh[None, :]
    hard = (gated > 0).astype(np.float32)
    inactive = hard.sum(axis=1) == 0
    mask = hard
    if inactive.any():
        li = logits[inactive]
        topk_idx = np.argsort(-li, axis=1, kind="stable")[:, :k]
        fallback = np.zeros_like(li)
        np.put_along_axis(fallback, topk_idx, 1.0, axis=1)
        mask[inactive] = fallback
    return mask, logits, res


def kernel(hidden_states, sim_matrix, gates, temperature, experts_mask,
           min_experts_per_tok):
    k = int(np.asarray(min_experts_per_tok))
    if not (1 <= k <= E):
        flat = np.asarray(hidden_states, dtype=np.float32).reshape(N, C)
        return _np_reference(
            flat,
            np.asarray(sim_matrix, dtype=np.float32),
            np.asarray(gates, dtype=np.float32),
            np.asarray(temperature, dtype=np.float32),
            np.asarray(experts_mask, dtype=np.float32),
            k,
        )
    xhs, shs, hctx = _prep_inputs(
        hidden_states, sim_matrix, gates, temperature, experts_mask
    )
    mask, logits, _ = run_on_device(xhs, shs, hctx, k)
    return mask, logits
